# revision 7
# baseline (speedup 1.0000x reference)
"""Bass/Trainium2 kernel for APPNP-with-GCN GNN message passing.

Algorithm (reference):
    src,dst = edges + self loops;  norm = dinv[src]*dinv[dst]  (deg over dst)
    h = relu(A(h@W)+b) x3 ;  APPNP: z <- 0.9*A z + 0.1*z0 x10 ; log_softmax(h@fcw+fcb)

Key algebraic rewrite: APPNP loop is linear, so push fc_w through it:
    z0 = h3 @ fc_w ; z_{k+1} = (1-a) A z_k + a z0 ; logits = z_K + fc_b
propagations 4..13 run at 40 (padded 64) dims instead of 256.

Distribution: nodes/edges sharded by destination across 8 cores; full feature
table replicated per-core via chunked AllGather between propagations; gathers
via indirect DMA; segment-sum via one-hot-matrix matmul in PSUM.
"""

import math
import numpy as np

import concourse.bass as bass
import concourse.bacc as bacc
import concourse.tile as tile
import concourse.mybir as mybir
from concourse.masks import make_identity

P = 128
ALPHA = 0.1
K_STEPS = 10
N_CORES = 8


# ---------------------------------------------------------------- geometry
class Geo:
    def __init__(self, n_nodes, d_in, d_hid, n_cls):
        assert d_in == d_hid
        self.n = n_nodes
        self.d = d_in                      # 256
        self.c = n_cls                     # 40
        self.cp = 64                       # padded class dim
        self.rows = (n_nodes + N_CORES - 1) // N_CORES   # real rows per core
        self.tiles = (self.rows + P - 1) // P            # dst tiles per core
        self.shard = self.tiles * P                      # padded rows per core
        # chunks for the allgather pipeline: largest divisor of tiles <= 8
        self.chunks = 1
        for cdiv in range(min(8, self.tiles), 0, -1):
            if self.tiles % cdiv == 0:
                self.chunks = cdiv
                break
        self.tiles_per_chunk = self.tiles // self.chunks
        self.chunk_rows = self.tiles_per_chunk * P
        self.tab_rows = N_CORES * self.shard

    def table_row(self, node):
        """Permuted full-table row index for node ids (vectorized)."""
        c = node // self.rows
        r = node - c * self.rows
        k = r // self.chunk_rows
        return k * (N_CORES * self.chunk_rows) + c * self.chunk_rows + (r - k * self.chunk_rows)


# ------------------------------------------------------------- host preproc
def preprocess(geo, edge_index):
    """Partition+sort edges by destination, build per-core meta arrays.

    Returns (B, metas) where metas[c] = dict(idx int32, slot f16, norm f16),
    each shaped [128, tiles*B] partition-major.
    """
    n = geo.n
    src = np.concatenate([edge_index[0], np.arange(n, dtype=np.int64)])
    dst = np.concatenate([edge_index[1], np.arange(n, dtype=np.int64)])
    deg = np.bincount(dst, minlength=n).astype(np.float64)
    dinv = 1.0 / np.sqrt(np.maximum(deg, 1.0))
    dinv[deg == 0] = 0.0
    norm = (dinv[src] * dinv[dst]).astype(np.float32)
    src_row = geo.table_row(src).astype(np.int32)

    core = dst // geo.rows
    dst_local = dst - core * geo.rows
    tile_id = dst_local // P
    slot = (dst_local - tile_id * P).astype(np.float32)

    # global sort by (core, tile) once
    order = np.argsort(core * geo.tiles + tile_id, kind="stable")
    core_s = core[order]
    tile_s = tile_id[order]
    slot_s = slot[order]
    norm_s = norm[order]
    srcr_s = src_row[order]

    key = core_s * geo.tiles + tile_s
    n_keys = N_CORES * geo.tiles
    counts = np.bincount(key, minlength=n_keys)
    B = int((counts.max() + P - 1) // P)
    starts = np.zeros(n_keys, dtype=np.int64)
    np.cumsum(counts[:-1], out=starts[1:])

    # scatter edges into padded [n_keys, B*128] arrays
    pos_in_key = np.arange(len(key)) - starts[key]
    flat_pos = key * (B * P) + pos_in_key
    idx_pad = np.zeros(n_keys * B * P, dtype=np.int32)
    slot_pad = np.zeros(n_keys * B * P, dtype=np.float16)
    norm_pad = np.zeros(n_keys * B * P, dtype=np.float16)
    idx_pad[flat_pos] = srcr_s
    slot_pad[flat_pos] = slot_s.astype(np.float16)
    norm_pad[flat_pos] = norm_s.astype(np.float16)

    metas = []
    for c in range(N_CORES):
        sl = slice(c * geo.tiles * B * P, (c + 1) * geo.tiles * B * P)
        # [tiles*B, 128] -> [128, tiles*B]
        i = np.ascontiguousarray(idx_pad[sl].reshape(geo.tiles * B, P).T)
        s = np.ascontiguousarray(slot_pad[sl].reshape(geo.tiles * B, P).T)
        v = np.ascontiguousarray(norm_pad[sl].reshape(geo.tiles * B, P).T)
        metas.append({"midx": i, "mslot": s, "mnorm": v})
    return B, metas


def build_x_table(geo, x):
    """Full permuted fp16 x table, replicated to every core."""
    xt = np.zeros((geo.tab_rows, geo.d), dtype=np.float16)
    rows = geo.table_row(np.arange(geo.n))
    xt[rows] = x.astype(np.float16)
    return xt


# ------------------------------------------------------------- bass builder
F16 = mybir.dt.float16
F32 = mybir.dt.float32
I32 = mybir.dt.int32


def ap3(base_ap, dims):
    """Manually shaped AP sharing base tensor/offset."""
    return bass.AP(base_ap.tensor, base_ap.offset, dims)


def build_program(geo, B):
    nc = bacc.Bacc(None, target_bir_lowering=False, num_devices=N_CORES)
    g = geo
    NB = g.tiles * B
    D, CP = g.d, g.cp
    RG = [list(range(N_CORES))]

    # ---------------- parameters
    xtab_p = nc.declare_dram_parameter("xtab", [g.tab_rows, D], F16, isOutput=False)
    midx_p = nc.declare_dram_parameter("midx", [P, NB], I32, isOutput=False)
    mslot_p = nc.declare_dram_parameter("mslot", [P, NB], F16, isOutput=False)
    mnorm_p = nc.declare_dram_parameter("mnorm", [P, NB], F16, isOutput=False)
    w_p = [nc.declare_dram_parameter(f"w{i+1}", [D, D], F16, isOutput=False) for i in range(3)]
    b_p = [nc.declare_dram_parameter(f"b{i+1}", [P, D], F32, isOutput=False) for i in range(3)]
    fcw_p = nc.declare_dram_parameter("fcw", [D, CP], F16, isOutput=False)
    fcb_p = nc.declare_dram_parameter("fcb", [P, CP], F32, isOutput=False)
    out_p = nc.declare_dram_parameter("out", [g.shard, g.c], F32, isOutput=True)

    iota_np = np.tile(np.arange(P, dtype=np.float16), (P, 1))
    iota_dram = nc.inline_tensor(iota_np, name="iota_const")

    with tile.TileContext(nc) as tc:
        with (
            tc.tile_pool(name="const", bufs=1) as constp,
            tc.tile_pool(name="meta", bufs=1) as metap,
            tc.tile_pool(name="dram", bufs=1, space="DRAM") as dramp,
            tc.tile_pool(name="dramsh", bufs=1, space="DRAM") as dramshp,
            tc.tile_pool(name="msg", bufs=3) as msgp,
            tc.tile_pool(name="sel", bufs=3) as selp,
            tc.tile_pool(name="stage", bufs=3) as stagep,
            tc.tile_pool(name="psum", bufs=2, space="PSUM") as psp,
        ):
            # ---------------- constants / resident state
            iota_sb = constp.tile([P, P], F16, tag="iota")
            nc.sync.dma_start(out=iota_sb[:], in_=iota_dram[:, :])
            ident = constp.tile([P, P], F16, tag="ident")
            make_identity(nc, ident[:])

            idx_sb = metap.tile([P, NB], I32, tag="midx")
            slot_sb = metap.tile([P, NB], F16, tag="mslot")
            norm_sb = metap.tile([P, NB], F16, tag="mnorm")
            nc.sync.dma_start(out=idx_sb[:], in_=midx_p[:, :])
            nc.sync.dma_start(out=slot_sb[:], in_=mslot_p[:, :])
            nc.sync.dma_start(out=norm_sb[:], in_=mnorm_p[:, :])

            w_sb = []
            for i in range(3):
                w = constp.tile([P, 2 * D], F16, tag=f"w{i}", name=f"wsb{i}")
                nc.sync.dma_start(out=w[:, 0:D], in_=w_p[i][0:P, :])
                nc.sync.dma_start(out=w[:, D:2 * D], in_=w_p[i][P:2 * P, :])
                w_sb.append(w)
            b_sb = []
            for i in range(3):
                b = constp.tile([P, D], F32, tag=f"b{i}", name=f"bsb{i}")
                nc.sync.dma_start(out=b[:], in_=b_p[i][:, :])
                b_sb.append(b)
            fcw_sb = constp.tile([P, 2 * CP], F16, tag="fcw")
            nc.sync.dma_start(out=fcw_sb[:, 0:CP], in_=fcw_p[0:P, :])
            nc.sync.dma_start(out=fcw_sb[:, CP:2 * CP], in_=fcw_p[P:2 * P, :])
            fcb_sb = constp.tile([P, CP], F32, tag="fcb")
            nc.sync.dma_start(out=fcb_sb[:], in_=fcb_p[:, :])

            z0s_sb = metap.tile([P, g.tiles * CP], F32, tag="z0s")  # alpha * z0

            # ---------------- DRAM tables / bounces
            tabD = [dramshp.tile([g.tab_rows, D], F16, tag=f"tabD{i}", name=f"tabD{i}") for i in range(2)]
            tabZ = [dramshp.tile([g.tab_rows, CP], F16, tag=f"tabZ{i}", name=f"tabZ{i}") for i in range(2)]
            aginD = dramp.tile([g.shard, D], F16, tag="aginD")
            aginZ = dramp.tile([g.shard, CP], F16, tag="aginZ")

            def build_S(t, scale_norm):
                """S [128, B*128] one-hot (optionally norm-scaled) for tile t."""
                seq = selp.tile([P, B * P], F16, tag="seq")
                sl = slot_sb[:, t * B:(t + 1) * B].to_broadcast([P, B, P])
                io = ap3(iota_sb[:], [iota_sb[:].ap[0], [0, B], [1, P]])
                s3 = ap3(seq[:], [seq[:].ap[0], [P, B], [1, P]])
                nc.vector.tensor_tensor(out=s3, in0=sl, in1=io,
                                        op=mybir.AluOpType.is_equal)
                if not scale_norm:
                    return seq
                s = selp.tile([P, B * P], F16, tag="s")
                nm = norm_sb[:, t * B:(t + 1) * B].to_broadcast([P, B, P])
                nc.vector.tensor_tensor(out=ap3(s[:], [s[:].ap[0], [P, B], [1, P]]),
                                        in0=s3, in1=nm, op=mybir.AluOpType.mult)
                return s

            def gather(src_tab, t, d):
                msgs = msgp.tile([P, B * d], F16, tag=f"msg{d}")
                nc.gpsimd.indirect_dma_start(
                    out=msgs[:],
                    out_offset=None,
                    in_=src_tab[:, :],
                    in_offset=bass.IndirectOffsetOnAxis(
                        ap=idx_sb[:, t * B:(t + 1) * B], axis=0),
                )
                return msgs

            def allgather(bounce, tab, j, d):
                r0, r1 = j * g.chunk_rows, (j + 1) * g.chunk_rows
                o0, o1 = j * N_CORES * g.chunk_rows, (j + 1) * N_CORES * g.chunk_rows
                nc.gpsimd.collective_compute(
                    "AllGather", mybir.AluOpType.bypass, replica_groups=RG,
                    ins=[bounce[r0:r1, :]], outs=[tab[o0:o1, :]])

            # ================= GCN conv layers =================
            for l in range(3):
                src_tab = tabD[(l + 1) % 2] if l > 0 else None
                for t in range(g.tiles):
                    if l == 0:
                        msgs = gather(xtab_p, t, D)
                    else:
                        msgs = gather(src_tab, t, D)
                    S = build_S(t, scale_norm=True)
                    pA = psp.tile([P, D], F32, tag="pA", space="PSUM")
                    for b in range(B):
                        nc.tensor.matmul(
                            out=pA[:], lhsT=S[:, b * P:(b + 1) * P],
                            rhs=msgs[:, b * D:(b + 1) * D],
                            start=(b == 0), stop=(b == B - 1))
                    # A@h tile (node-major) -> fp16
                    ah = stagep.tile([P, D], F16, tag="ah")
                    nc.vector.tensor_copy(out=ah[:], in_=pA[:])
                    # transpose halves -> (A@h)^T
                    ahT = stagep.tile([P, D], F16, tag="ahT")
                    for h in range(2):
                        pT = psp.tile([P, P], F16, tag="pT", space="PSUM")
                        nc.tensor.transpose(out=pT[:], in_=ah[:, h * P:(h + 1) * P],
                                            identity=ident[:])
                        nc.vector.tensor_copy(out=ahT[:, h * P:(h + 1) * P], in_=pT[:])
                    # transform: psum2 = (A@h) @ W
                    p2 = psp.tile([P, D], F32, tag="p2", space="PSUM")
                    for h in range(2):
                        nc.tensor.matmul(out=p2[:], lhsT=ahT[:, h * P:(h + 1) * P],
                                         rhs=w_sb[l][:, h * D:(h + 1) * D],
                                         start=(h == 0), stop=(h == 1))
                    # epilogue: h = relu(p2 + b)
                    tmp = stagep.tile([P, D], F32, tag="tmp32")
                    nc.vector.tensor_tensor(out=tmp[:], in0=p2[:], in1=b_sb[l][:],
                                            op=mybir.AluOpType.add)
                    hrow = stagep.tile([P, D], F16, tag="hrow")
                    nc.scalar.activation(out=hrow[:], in_=tmp[:],
                                         func=mybir.ActivationFunctionType.Relu)
                    if l < 2:
                        nc.sync.dma_start(out=aginD[t * P:(t + 1) * P, :], in_=hrow[:])
                    else:
                        # z0 = h3 @ fcw : transpose h3 then project
                        h3T = stagep.tile([P, D], F16, tag="h3T")
                        for h in range(2):
                            pT2 = psp.tile([P, P], F16, tag="pT", space="PSUM", name="pT2")
                            nc.tensor.transpose(out=pT2[:], in_=hrow[:, h * P:(h + 1) * P],
                                                identity=ident[:])
                            nc.vector.tensor_copy(out=h3T[:, h * P:(h + 1) * P], in_=pT2[:])
                        p4 = psp.tile([P, D], F32, tag="p2", space="PSUM", name="p4")
                        for h in range(2):
                            nc.tensor.matmul(out=p4[:, :CP], lhsT=h3T[:, h * P:(h + 1) * P],
                                             rhs=fcw_sb[:, h * CP:(h + 1) * CP],
                                             start=(h == 0), stop=(h == 1))
                        z0row = stagep.tile([P, CP], F16, tag="z0row")
                        nc.vector.tensor_copy(out=z0row[:], in_=p4[:, :CP])
                        nc.vector.tensor_scalar_mul(
                            out=z0s_sb[:, t * CP:(t + 1) * CP], in0=p4[:, :CP], scalar1=ALPHA)
                        nc.sync.dma_start(out=aginZ[t * P:(t + 1) * P, :], in_=z0row[:])
                    if (t + 1) % g.tiles_per_chunk == 0:
                        j = (t + 1) // g.tiles_per_chunk - 1
                        if l < 2:
                            allgather(aginD, tabD[l % 2], j, D)
                        else:
                            allgather(aginZ, tabZ[0], j, CP)

            # ================= APPNP steps =================
            for k in range(K_STEPS):
                src_tab = tabZ[k % 2]
                dst_tab = tabZ[(k + 1) % 2]
                last = (k == K_STEPS - 1)
                for t in range(g.tiles):
                    msgs = gather(src_tab, t, CP)
                    S = build_S(t, scale_norm=False)
                    # scale messages by norm (cheaper than scaling S at CP<128)
                    msc = msgp.tile([P, B * CP], F16, tag="msc")
                    nm = norm_sb[:, t * B:(t + 1) * B].to_broadcast([P, B, CP])
                    nc.vector.tensor_tensor(
                        out=ap3(msc[:], [msc[:].ap[0], [CP, B], [1, CP]]),
                        in0=ap3(msgs[:], [msgs[:].ap[0], [CP, B], [1, CP]]),
                        in1=nm, op=mybir.AluOpType.mult)
                    p5 = psp.tile([P, D], F32, tag="pA", space="PSUM", name="p5")
                    for b in range(B):
                        nc.tensor.matmul(
                            out=p5[:, :CP], lhsT=S[:, b * P:(b + 1) * P],
                            rhs=msc[:, b * CP:(b + 1) * CP],
                            start=(b == 0), stop=(b == B - 1))
                    # z = (1-a)*p5 + z0s   (z0s pre-scaled by a)
                    zt = stagep.tile([P, CP], F32, tag="zt32")
                    nc.vector.tensor_scalar_mul(out=zt[:], in0=p5[:, :CP], scalar1=1.0 - ALPHA)
                    if not last:
                        zrow = stagep.tile([P, CP], F16, tag="zrow")
                        nc.vector.tensor_tensor(out=zrow[:], in0=zt[:],
                                                in1=z0s_sb[:, t * CP:(t + 1) * CP],
                                                op=mybir.AluOpType.add)
                        nc.sync.dma_start(out=aginZ[t * P:(t + 1) * P, :], in_=zrow[:])
                        if (t + 1) % g.tiles_per_chunk == 0:
                            j = (t + 1) // g.tiles_per_chunk - 1
                            allgather(aginZ, dst_tab, j, CP)
                    else:
                        lg = stagep.tile([P, CP], F32, tag="lg")
                        nc.vector.tensor_tensor(out=lg[:], in0=zt[:],
                                                in1=z0s_sb[:, t * CP:(t + 1) * CP],
                                                op=mybir.AluOpType.add)
                        lgb = stagep.tile([P, CP], F32, tag="lgb")
                        nc.vector.tensor_tensor(out=lgb[:], in0=lg[:], in1=fcb_sb[:],
                                                op=mybir.AluOpType.add)
                        # log_softmax over first c columns
                        mx = stagep.tile([P, 1], F32, tag="mx")
                        nc.vector.tensor_reduce(out=mx[:], in_=lgb[:, :g.c],
                                                axis=mybir.AxisListType.X,
                                                op=mybir.AluOpType.max)
                        nmx = stagep.tile([P, 1], F32, tag="nmx")
                        nc.vector.tensor_scalar_mul(out=nmx[:], in0=mx[:], scalar1=-1.0)
                        ex = stagep.tile([P, g.c], F32, tag="ex")
                        se = stagep.tile([P, 1], F32, tag="se")
                        nc.scalar.activation(out=ex[:], in_=lgb[:, :g.c],
                                             func=mybir.ActivationFunctionType.Exp,
                                             bias=nmx[:, :1], accum_out=se[:, :1])
                        lse = stagep.tile([P, 1], F32, tag="lse")
                        nc.scalar.activation(out=lse[:], in_=se[:],
                                             func=mybir.ActivationFunctionType.Ln)
                        res = stagep.tile([P, g.c], F32, tag="res")
                        nc.vector.tensor_scalar(
                            out=res[:], in0=lgb[:, :g.c],
                            scalar1=mx[:, :1], scalar2=lse[:, :1],
                            op0=mybir.AluOpType.subtract,
                            op1=mybir.AluOpType.subtract)
                        nc.sync.dma_start(out=out_p[t * P:(t + 1) * P, :], in_=res[:])
    return nc


# ------------------------------------------------------------- entry point
def make_in_maps(geo, B, metas, inputs):
    xt = build_x_table(geo, np.asarray(inputs["x"], dtype=np.float32))
    in_maps = []
    for c in range(N_CORES):
        m = {
            "xtab": xt,
            "midx": metas[c]["midx"],
            "mslot": metas[c]["mslot"],
            "mnorm": metas[c]["mnorm"],
            "fcw": np.zeros((geo.d, geo.cp), dtype=np.float16),
            "fcb": np.zeros((P, geo.cp), dtype=np.float32),
        }
        m["fcw"][:, :geo.c] = np.asarray(inputs["fc_w"], dtype=np.float16)
        m["fcb"][:, :geo.c] = np.tile(np.asarray(inputs["fc_b"], dtype=np.float32), (P, 1))
        for i in range(3):
            m[f"w{i+1}"] = np.asarray(inputs[f"w{i+1}"], dtype=np.float16)
            m[f"b{i+1}"] = np.tile(np.asarray(inputs[f"b{i+1}"], dtype=np.float32), (P, 1))
        in_maps.append(m)
    return in_maps


def assemble_output(geo, results):
    outs = [results[c]["out"][: geo.rows] for c in range(N_CORES)]
    return np.concatenate(outs, axis=0)[: geo.n].astype(np.float32)


_LAST_EXEC_NS = None


def kernel(x, edge_index, w1, b1, w2, b2, w3, b3, fc_w, fc_b):
    global _LAST_EXEC_NS
    from concourse.bass_utils import run_bass_kernel_spmd

    geo = Geo(x.shape[0], x.shape[1], w1.shape[1], fc_w.shape[1])
    B, metas = preprocess(geo, np.asarray(edge_index))
    nc = build_program(geo, B)
    nc.compile()
    inputs = dict(x=x, w1=w1, b1=b1, w2=w2, b2=b2, w3=w3, b3=b3,
                  fc_w=fc_w, fc_b=fc_b)
    in_maps = make_in_maps(geo, B, metas, inputs)
    try:
        res = run_bass_kernel_spmd(nc, in_maps, core_ids=list(range(N_CORES)),
                                   trace=True)
    except ModuleNotFoundError:
        res = run_bass_kernel_spmd(nc, in_maps, core_ids=list(range(N_CORES)),
                                   trace=False)
    _LAST_EXEC_NS = res.exec_time_ns
    return assemble_output(geo, res.results)


# revision 11
# speedup vs baseline: 1.0152x; 1.0152x over previous
"""Bass/Trainium2 kernel for APPNP-with-GCN GNN message passing.

Algorithm (reference):
    src,dst = edges + self loops;  norm = dinv[src]*dinv[dst]  (deg over dst)
    h = relu(A(h@W)+b) x3 ;  APPNP: z <- 0.9*A z + 0.1*z0 x10 ; log_softmax(h@fcw+fcb)

Algebraic rewrites:
  1. APPNP loop is linear -> push fc_w through it: z0 = h3@fc_w and iterate in
     40(pad 64)-dim space.
  2. dinv-folding: tables store u = dinv*h. Then propagation is a plain
     segment-sum of gathered u rows (no per-edge norm!), with dinv^2 / dinv
     applied as per-dst-row scalars in the epilogue.

Distribution: nodes/edges sharded by destination across 8 cores; u-tables
replicated per-core via chunked AllGather between propagations; gathers via
indirect DMA; segment-sum via one-hot matmul in PSUM (S built on-device from
slot metadata; pad lanes use slot=-1 so they contribute zero).
"""

import numpy as np

import concourse.bass as bass
import concourse.bacc as bacc
import concourse.tile as tile
import concourse.mybir as mybir

P = 128
ALPHA = 0.1
K_STEPS = 10
N_CORES = 8


# ---------------------------------------------------------------- geometry
class Geo:
    def __init__(self, n_nodes, d_in, d_hid, n_cls):
        assert d_in == d_hid
        self.n = n_nodes
        self.d = d_in                      # 256
        self.c = n_cls                     # 40
        self.cp = ((n_cls + 7) // 8) * 8   # padded class dim (40)
        self.rows = (n_nodes + N_CORES - 1) // N_CORES   # real rows per core
        self.tiles = (self.rows + P - 1) // P            # dst tiles per core
        self.shard = self.tiles * P                      # padded rows per core
        self.chunks = 2 if self.tiles % 2 == 0 else 1
        self.tiles_per_chunk = self.tiles // self.chunks
        self.chunk_rows = self.tiles_per_chunk * P
        self.tab_rows = N_CORES * self.shard

    def table_row(self, node):
        """Permuted full-table row index for node ids (vectorized)."""
        c = node // self.rows
        r = node - c * self.rows
        k = r // self.chunk_rows
        return k * (N_CORES * self.chunk_rows) + c * self.chunk_rows + (r - k * self.chunk_rows)


# ------------------------------------------------------------- host preproc
def preprocess(geo, edge_index):
    """Partition+sort edges by destination, build per-core meta arrays.

    Returns (B, dinv, metas); metas[c] = dict(midx int32 [128,tiles*B],
    mslot f16 [128,tiles*B] (-1 on pads), dcols f32 [128, tiles*4]).
    """
    n = geo.n
    src = np.concatenate([edge_index[0], np.arange(n, dtype=np.int64)])
    dst = np.concatenate([edge_index[1], np.arange(n, dtype=np.int64)])
    deg = np.bincount(dst, minlength=n).astype(np.float64)
    dinv = 1.0 / np.sqrt(np.maximum(deg, 1.0))
    dinv[deg == 0] = 0.0
    src_row = geo.table_row(src).astype(np.int32)

    core = dst // geo.rows
    dst_local = dst - core * geo.rows
    tile_id = dst_local // P
    slot = (dst_local - tile_id * P).astype(np.float32)

    order = np.argsort(core * geo.tiles + tile_id, kind="stable")
    key = (core * geo.tiles + tile_id)[order]
    slot_s = slot[order]
    srcr_s = src_row[order]

    n_keys = N_CORES * geo.tiles
    counts = np.bincount(key, minlength=n_keys)
    B = int((counts.max() + P - 1) // P)
    starts = np.zeros(n_keys, dtype=np.int64)
    np.cumsum(counts[:-1], out=starts[1:])

    pos_in_key = np.arange(len(key)) - starts[key]
    flat_pos = key * (B * P) + pos_in_key
    idx_pad = np.zeros(n_keys * B * P, dtype=np.int32)
    slot_pad = np.full(n_keys * B * P, -1.0, dtype=np.float16)
    idx_pad[flat_pos] = srcr_s
    slot_pad[flat_pos] = slot_s.astype(np.float16)

    metas = []
    for c in range(N_CORES):
        sl = slice(c * geo.tiles * B * P, (c + 1) * geo.tiles * B * P)
        i = np.ascontiguousarray(idx_pad[sl].reshape(geo.tiles * B, P).T)
        s = np.ascontiguousarray(slot_pad[sl].reshape(geo.tiles * B, P).T)
        # per-dst-row scalar columns: [dinv2, dinv, rdinv, 0] per tile
        dv = np.zeros(geo.shard, dtype=np.float64)
        base = c * geo.rows
        real = min(geo.rows, max(0, n - base))
        dv[:real] = dinv[base:base + real]
        dcols = np.zeros((P, geo.tiles * 4), dtype=np.float32)
        dvt = dv.reshape(geo.tiles, P)
        for t in range(geo.tiles):
            dcols[:, 4 * t + 0] = (dvt[t] ** 2).astype(np.float32)
            dcols[:, 4 * t + 1] = dvt[t].astype(np.float32)
            rd = np.where(dvt[t] > 0, 1.0 / np.maximum(dvt[t], 1e-30), 0.0)
            dcols[:, 4 * t + 2] = rd.astype(np.float32)
        metas.append({"midx": i, "mslot": s, "dcols": dcols})
    return B, dinv, metas


def build_x_table(geo, x, dinv):
    """Full permuted fp16 u-table for x (u = dinv*x), replicated per core."""
    xt = np.zeros((geo.tab_rows, geo.d), dtype=np.float16)
    rows = geo.table_row(np.arange(geo.n))
    xt[rows] = (x.astype(np.float64) * dinv[:, None]).astype(np.float16)
    return xt


# ------------------------------------------------------------- bass builder
F16 = mybir.dt.float16
F32 = mybir.dt.float32
I32 = mybir.dt.int32
ACTF = mybir.ActivationFunctionType
OP = mybir.AluOpType


def ap3(base_ap, dims):
    return bass.AP(base_ap.tensor, base_ap.offset, dims)


def build_program(geo, B, has_bias=(False, False, False), has_fcb=False):
    nc = bacc.Bacc(None, target_bir_lowering=False, num_devices=N_CORES)
    g = geo
    NB = g.tiles * B
    D, CP = g.d, g.cp
    RG = [list(range(N_CORES))]

    # ---------------- parameters
    xtab_p = nc.declare_dram_parameter("xtab", [g.tab_rows, D], F16, isOutput=False)
    midx_p = nc.declare_dram_parameter("midx", [P, NB], I32, isOutput=False)
    mslot_p = nc.declare_dram_parameter("mslot", [P, NB], F16, isOutput=False)
    dcols_p = nc.declare_dram_parameter("dcols", [P, g.tiles * 4], F32, isOutput=False)
    w_p = [nc.declare_dram_parameter(f"w{i+1}", [D, D], F16, isOutput=False) for i in range(3)]
    bdt_p = [nc.declare_dram_parameter(f"bdt{i+1}", [g.shard, D], F32, isOutput=False)
             if has_bias[i] else None for i in range(3)]
    fcw_p = nc.declare_dram_parameter("fcw", [D, CP], F16, isOutput=False)
    fcb_p = nc.declare_dram_parameter("fcb", [P, CP], F32, isOutput=False)
    out_p = nc.declare_dram_parameter("out", [g.shard, g.c], F32, isOutput=True)

    # constants: iota (block-major) and iotaB (d-major, for the GCN S layout)
    iota_np = np.tile(np.arange(P, dtype=np.float16), (P, 1))
    iota_dram = nc.inline_tensor(iota_np, name="iota_const")
    iotaB_np = np.tile(np.repeat(np.arange(P, dtype=np.float16), B), (P, 1))
    iotaB_dram = nc.inline_tensor(iotaB_np, name="iotaB_const")

    with tile.TileContext(nc) as tc:
        with (
            tc.tile_pool(name="const", bufs=1) as constp,
            tc.tile_pool(name="meta", bufs=1) as metap,
            tc.tile_pool(name="dram", bufs=1, space="DRAM") as dramp,
            tc.tile_pool(name="msg", bufs=3) as msgp,
            tc.tile_pool(name="sel", bufs=3) as selp,
            tc.tile_pool(name="stage", bufs=3) as stagep,
            tc.tile_pool(name="psum", bufs=2, space="PSUM") as psp,
        ):
            # ---------------- constants / resident state
            iota_sb = constp.tile([P, P], F16, tag="iota")
            nc.sync.dma_start(out=iota_sb[:], in_=iota_dram[:, :])
            iotaB_sb = constp.tile([P, P * B], F16, tag="iotaB")
            nc.sync.dma_start(out=iotaB_sb[:], in_=iotaB_dram[:, :])
            idn = constp.tile([P, P], F16, tag="idn")
            from concourse.masks import make_identity
            make_identity(nc, idn[:])

            idx_sb = metap.tile([P, NB], I32, tag="midx")
            slot_sb = metap.tile([P, NB], F16, tag="mslot")
            dcol_sb = metap.tile([P, g.tiles * 4], F32, tag="dcols")
            nc.sync.dma_start(out=idx_sb[:], in_=midx_p[:, :])
            nc.sync.dma_start(out=slot_sb[:], in_=mslot_p[:, :])
            nc.sync.dma_start(out=dcol_sb[:], in_=dcols_p[:, :])

            w_sb = []
            for i in range(3):
                w = constp.tile([P, 2 * D], F16, tag=f"w{i}", name=f"wsb{i}")
                nc.sync.dma_start(out=w[:, 0:D], in_=w_p[i][0:P, :])
                nc.sync.dma_start(out=w[:, D:2 * D], in_=w_p[i][P:2 * P, :])
                w_sb.append(w)
            fcw_sb = constp.tile([P, 2 * CP], F16, tag="fcw")
            nc.sync.dma_start(out=fcw_sb[:, 0:CP], in_=fcw_p[0:P, :])
            nc.sync.dma_start(out=fcw_sb[:, CP:2 * CP], in_=fcw_p[P:2 * P, :])
            fcb_sb = constp.tile([P, CP], F32, tag="fcb")
            nc.sync.dma_start(out=fcb_sb[:], in_=fcb_p[:, :])

            z0s_sb = metap.tile([P, g.tiles * CP], F32, tag="z0s")  # 0.1*dinv*z0

            # ---------------- DRAM tables / bounces
            tabD = [dramp.tile([g.tab_rows, D], F16, tag=f"tabD{i}", name=f"tabD{i}")
                    for i in range(2)]
            tabZ = [dramp.tile([g.tab_rows, CP], F16, tag=f"tabZ{i}", name=f"tabZ{i}")
                    for i in range(2)]
            aginD = dramp.tile([g.shard, D], F16, tag="aginD")
            aginZ = dramp.tile([g.shard, CP], F16, tag="aginZ")

            def dcol(t, j):
                return dcol_sb[:, 4 * t + j:4 * t + j + 1]

            def gather(src_tab, t, d):
                msgs = msgp.tile([P, B * d], F16, tag=f"msg{d}")
                nc.gpsimd.indirect_dma_start(
                    out=msgs[:], out_offset=None,
                    in_=src_tab[:, :],
                    in_offset=bass.IndirectOffsetOnAxis(
                        ap=idx_sb[:, t * B:(t + 1) * B], axis=0),
                )
                return msgs

            def allgather(bounce, tab, j, d):
                r0, r1 = j * g.chunk_rows, (j + 1) * g.chunk_rows
                o0, o1 = j * N_CORES * g.chunk_rows, (j + 1) * N_CORES * g.chunk_rows
                nc.gpsimd.collective_compute(
                    "AllGather", OP.bypass, replica_groups=RG,
                    ins=[bounce[r0:r1, :]], outs=[tab[o0:o1, :]])

            # ================= GCN conv layers =================
            for l in range(3):
                src_tab = xtab_p if l == 0 else tabD[(l + 1) % 2]
                last_gcn = (l == 2)
                for t in range(g.tiles):
                    msgs = gather(src_tab, t, D)
                    # S' d-major [128, d*B+b] via 2x-mode is_equal
                    sp = selp.tile([P, P * B], F16, tag="sp")
                    sl3 = ap3(slot_sb[:, t * B:(t + 1) * B],
                              [slot_sb[:].ap[0], [0, P], [1, B]])
                    io3 = ap3(iotaB_sb[:], [iotaB_sb[:].ap[0], [B, P], [1, B]])
                    sp3 = ap3(sp[:], [sp[:].ap[0], [B, P], [1, B]])
                    nc.vector.tensor_tensor(out=sp3, in0=sl3, in1=io3, op=OP.is_equal)
                    # segment-sum, transposed out: psumT[h][f,d] = sum_e msg[e,f]*S'[e,d]
                    pT = [psp.tile([P, P], F32, tag=f"pT{h}", space="PSUM",
                                   name=f"pT{h}", bufs=1) for h in range(2)]
                    for b in range(B):
                        base = sp[:]
                        rhs = bass.AP(base.tensor, base.offset + b,
                                      [base.ap[0], [B, P]])
                        for h in range(2):
                            nc.tensor.matmul(
                                out=pT[h][:],
                                lhsT=msgs[:, b * D + h * P:b * D + (h + 1) * P],
                                rhs=rhs, start=(b == 0), stop=(b == B - 1))
                    ahT = stagep.tile([P, D], F16, tag="ahT")
                    for h in range(2):
                        nc.scalar.copy(out=ahT[:, h * P:(h + 1) * P], in_=pT[h][:])
                    # transform: p2 = (A@h) @ W   [node-major]
                    p2 = psp.tile([P, D], F32, tag="p2", space="PSUM", bufs=1)
                    for h in range(2):
                        nc.tensor.matmul(out=p2[:], lhsT=ahT[:, h * P:(h + 1) * P],
                                         rhs=w_sb[l][:, h * D:(h + 1) * D],
                                         start=(h == 0), stop=(h == 1))
                    # epilogue: u/h = relu(scale*p2 + bias)
                    scale = dcol(t, 1) if last_gcn else dcol(t, 0)
                    if has_bias[l]:
                        s1 = stagep.tile([P, D], F32, tag="s1")
                        nc.scalar.activation(out=s1[:], in_=p2[:], func=ACTF.Copy,
                                             scale=scale)
                        bt = stagep.tile([P, D], F32, tag="bt")
                        nc.sync.dma_start(out=bt[:],
                                          in_=bdt_p[l][t * P:(t + 1) * P, :])
                        s2 = stagep.tile([P, D], F32, tag="s2")
                        nc.vector.tensor_tensor(out=s2[:], in0=s1[:], in1=bt[:],
                                                op=OP.add)
                        hrow = stagep.tile([P, D], F16, tag="hrow")
                        nc.scalar.activation(out=hrow[:], in_=s2[:], func=ACTF.Relu)
                    else:
                        hrow = stagep.tile([P, D], F16, tag="hrow")
                        nc.scalar.activation(out=hrow[:], in_=p2[:], func=ACTF.Relu,
                                             scale=scale)
                    if not last_gcn:
                        nc.sync.dma_start(out=aginD[t * P:(t + 1) * P, :], in_=hrow[:])
                    else:
                        # z0 = h3 @ fcw : transpose h3 (PE) then project
                        h3T = stagep.tile([P, D], F16, tag="h3T")
                        for h in range(2):
                            pTr = psp.tile([P, P], F16, tag="pTr", space="PSUM", bufs=1)
                            nc.tensor.transpose(out=pTr[:],
                                                in_=hrow[:, h * P:(h + 1) * P],
                                                identity=idn[:])
                            nc.scalar.copy(out=h3T[:, h * P:(h + 1) * P], in_=pTr[:])
                        p4 = psp.tile([P, CP], F32, tag="p4", space="PSUM", bufs=1)
                        for h in range(2):
                            nc.tensor.matmul(out=p4[:], lhsT=h3T[:, h * P:(h + 1) * P],
                                             rhs=fcw_sb[:, h * CP:(h + 1) * CP],
                                             start=(h == 0), stop=(h == 1))
                        z0row = stagep.tile([P, CP], F16, tag="z0row")
                        nc.scalar.activation(out=z0row[:], in_=p4[:], func=ACTF.Copy,
                                             scale=dcol(t, 1))  # dinv*z0
                        nc.vector.tensor_scalar(
                            out=z0s_sb[:, t * CP:(t + 1) * CP], in0=p4[:],
                            scalar1=dcol(t, 1), scalar2=ALPHA,
                            op0=OP.mult, op1=OP.mult)  # 0.1*dinv*z0
                        nc.sync.dma_start(out=aginZ[t * P:(t + 1) * P, :], in_=z0row[:])
                    if (t + 1) % g.tiles_per_chunk == 0:
                        j = (t + 1) // g.tiles_per_chunk - 1
                        if not last_gcn:
                            allgather(aginD, tabD[l % 2], j, D)
                        else:
                            allgather(aginZ, tabZ[0], j, CP)

            # ================= APPNP steps =================
            tstep = 2 if g.tiles % 2 == 0 else 1
            for k in range(K_STEPS):
                src_tab = tabZ[k % 2]
                dst_tab = tabZ[(k + 1) % 2]
                last = (k == K_STEPS - 1)
                for t0 in range(0, g.tiles, tstep):
                  # merged gather over tstep tiles (amortizes SWDGE fixed cost)
                  msgs2 = msgp.tile([P, tstep * B * CP], F16, tag="msgz")
                  nc.gpsimd.indirect_dma_start(
                      out=msgs2[:], out_offset=None,
                      in_=src_tab[:, :],
                      in_offset=bass.IndirectOffsetOnAxis(
                          ap=idx_sb[:, t0 * B:(t0 + tstep) * B], axis=0),
                  )
                  for t in range(t0, t0 + tstep):
                    moff = (t - t0) * B * CP
                    # d-major one-hot S' (2x-mode is_equal), shared w/ GCN shape
                    sp = selp.tile([P, P * B], F16, tag="sp", name="spz")
                    sl3 = ap3(slot_sb[:, t * B:(t + 1) * B],
                              [slot_sb[:].ap[0], [0, P], [1, B]])
                    io3 = ap3(iotaB_sb[:], [iotaB_sb[:].ap[0], [B, P], [1, B]])
                    sp3 = ap3(sp[:], [sp[:].ap[0], [B, P], [1, B]])
                    nc.vector.tensor_tensor(out=sp3, in0=sl3, in1=io3, op=OP.is_equal)
                    # transposed segment-sum: pT5[c,d] = sum_e msgs[e,c]*S'[e,d]
                    pT5 = psp.tile([CP, P], F32, tag="pT5", space="PSUM", bufs=1)
                    for b in range(B):
                        base = sp[:]
                        rhs = bass.AP(base.tensor, base.offset + b,
                                      [base.ap[0], [B, P]])
                        nc.tensor.matmul(
                            out=pT5[:],
                            lhsT=msgs2[:, moff + b * CP:moff + (b + 1) * CP],
                            rhs=rhs, start=(b == 0), stop=(b == B - 1))
                    aT16 = stagep.tile([CP, P], F16, tag="aT16")
                    nc.scalar.copy(out=aT16[:], in_=pT5[:])
                    pz = psp.tile([P, CP], F16, tag="pz", space="PSUM", bufs=1)
                    nc.tensor.transpose(out=pz[:], in_=aT16[:],
                                        identity=idn[:CP, :CP])
                    zt = stagep.tile([P, CP], F32, tag="zt32")
                    nc.vector.tensor_scalar(out=zt[:], in0=pz[:],
                                            scalar1=dcol(t, 0), scalar2=1.0 - ALPHA,
                                            op0=OP.mult, op1=OP.mult)
                    if not last:
                        # u_{k+1} = 0.9*dinv2*p5 + z0s
                        zrow = stagep.tile([P, CP], F16, tag="zrow")
                        nc.vector.tensor_tensor(out=zrow[:], in0=zt[:],
                                                in1=z0s_sb[:, t * CP:(t + 1) * CP],
                                                op=OP.add)
                        nc.sync.dma_start(out=aginZ[t * P:(t + 1) * P, :], in_=zrow[:])
                        if (t + 1) % g.tiles_per_chunk == 0:
                            j = (t + 1) // g.tiles_per_chunk - 1
                            allgather(aginZ, dst_tab, j, CP)
                    else:
                        # z_K = (0.9*dinv2*p5 + z0s) * rdinv ; logits; log_softmax
                        zu = stagep.tile([P, CP], F32, tag="zu")
                        nc.vector.tensor_tensor(out=zu[:], in0=zt[:],
                                                in1=z0s_sb[:, t * CP:(t + 1) * CP],
                                                op=OP.add)
                        lg = stagep.tile([P, CP], F32, tag="lg")
                        nc.vector.tensor_scalar(out=lg[:], in0=zu[:],
                                                scalar1=dcol(t, 2), scalar2=None,
                                                op0=OP.mult)
                        if has_fcb:
                            lgb = stagep.tile([P, CP], F32, tag="lgb")
                            nc.vector.tensor_tensor(out=lgb[:], in0=lg[:],
                                                    in1=fcb_sb[:], op=OP.add)
                        else:
                            lgb = lg
                        mx = stagep.tile([P, 1], F32, tag="mx")
                        nc.vector.tensor_reduce(out=mx[:], in_=lgb[:, :g.c],
                                                axis=mybir.AxisListType.X, op=OP.max)
                        nmx = stagep.tile([P, 1], F32, tag="nmx")
                        nc.vector.tensor_scalar_mul(out=nmx[:], in0=mx[:], scalar1=-1.0)
                        ex = stagep.tile([P, g.c], F32, tag="ex")
                        se = stagep.tile([P, 1], F32, tag="se")
                        nc.scalar.activation(out=ex[:], in_=lgb[:, :g.c],
                                             func=ACTF.Exp, bias=nmx[:, :1],
                                             accum_out=se[:, :1])
                        lse = stagep.tile([P, 1], F32, tag="lse")
                        nc.scalar.activation(out=lse[:], in_=se[:], func=ACTF.Ln)
                        res = stagep.tile([P, g.c], F32, tag="res")
                        nc.vector.tensor_scalar(
                            out=res[:], in0=lgb[:, :g.c],
                            scalar1=mx[:, :1], scalar2=lse[:, :1],
                            op0=OP.subtract, op1=OP.subtract)
                        nc.sync.dma_start(out=out_p[t * P:(t + 1) * P, :], in_=res[:])
    return nc


# ------------------------------------------------------------- entry point
def make_in_maps(geo, B, dinv, metas, inputs):
    xt = build_x_table(geo, np.asarray(inputs["x"], dtype=np.float32), dinv)
    has_bias = tuple(bool(np.any(np.asarray(inputs[f"b{i+1}"]))) for i in range(3))
    in_maps = []
    for c in range(N_CORES):
        m = {
            "xtab": xt,
            "midx": metas[c]["midx"],
            "mslot": metas[c]["mslot"],
            "dcols": metas[c]["dcols"],
            "fcw": np.zeros((geo.d, geo.cp), dtype=np.float16),
            "fcb": np.tile(np.pad(np.asarray(inputs["fc_b"], dtype=np.float32),
                                  (0, geo.cp - geo.c)), (P, 1)),
        }
        m["fcw"][:, :geo.c] = np.asarray(inputs["fc_w"], dtype=np.float16)
        for i in range(3):
            m[f"w{i+1}"] = np.asarray(inputs[f"w{i+1}"], dtype=np.float16)
            if has_bias[i]:
                b = np.asarray(inputs[f"b{i+1}"], dtype=np.float64)
                base = c * geo.rows
                dv = np.zeros(geo.shard, dtype=np.float64)
                real = min(geo.rows, max(0, geo.n - base))
                dv[:real] = dinv[base:base + real]
                if i < 2:
                    bdt = dv[:, None] * b[None, :]
                else:
                    bdt = np.tile(b[None, :], (geo.shard, 1))
                m[f"bdt{i+1}"] = bdt.astype(np.float32)
        in_maps.append(m)
    return in_maps


def assemble_output(geo, results):
    outs = [results[c]["out"][: geo.rows] for c in range(N_CORES)]
    return np.concatenate(outs, axis=0)[: geo.n].astype(np.float32)


_LAST_EXEC_NS = None


def kernel(x, edge_index, w1, b1, w2, b2, w3, b3, fc_w, fc_b):
    global _LAST_EXEC_NS
    from concourse.bass_utils import run_bass_kernel_spmd

    geo = Geo(x.shape[0], x.shape[1], w1.shape[1], fc_w.shape[1])
    B, dinv, metas = preprocess(geo, np.asarray(edge_index))
    has_bias = tuple(bool(np.any(np.asarray(b))) for b in (b1, b2, b3))
    has_fcb = bool(np.any(np.asarray(fc_b)))
    nc = build_program(geo, B, has_bias=has_bias, has_fcb=has_fcb)
    nc.compile()
    inputs = dict(x=x, w1=w1, b1=b1, w2=w2, b2=b2, w3=w3, b3=b3,
                  fc_w=fc_w, fc_b=fc_b)
    in_maps = make_in_maps(geo, B, dinv, metas, inputs)
    try:
        res = run_bass_kernel_spmd(nc, in_maps, core_ids=list(range(N_CORES)),
                                   trace=True)
    except ModuleNotFoundError:
        res = run_bass_kernel_spmd(nc, in_maps, core_ids=list(range(N_CORES)),
                                   trace=False)
    _LAST_EXEC_NS = res.exec_time_ns
    return assemble_output(geo, res.results)
